# revision 26
# baseline (speedup 1.0000x reference)
"""Mixtral-style sparse MoE block on 8 Trainium2 NeuronCores.

Strategy: 2D sharding — 4 FFN-quarters x 2 token-halves — all-bf16.
The router (tiny: T x H @ H x E) runs on the host as part of input
sharding. Each expert's routed tokens are split into two halves; cores
0-3 process half A and cores 4-7 process half B, with core (f, half)
computing only FFN columns [f*896, (f+1)*896). Every core therefore
processes ALL 8 experts over ~2048 tokens each, which balances the PE
load to within one token of the 8-core optimum regardless of routing
skew (the old expert-per-core layout ate the full max-expert imbalance,
~4%). The host sums the 4 FFN-quarter partial outputs per half, applies
the renormalized top-2 combine weights, and scatter-adds into [T, H].

Per-core program: 8 expert phases. Phase e holds w1/w3 (quarter, merged
into one [128, 7*2048] tile) and w2 (quarter, [128, 8*896]) in SBUF —
5.5 MB per expert, DMA'd once and prefetched one expert ahead — and
runs that expert's token slices:
    h  = silu(x @ w1q) * (x @ w3q)    # [C, 896]
    y += h @ w2q                      # [C, 1024] partial over F
in feature-on-partition layout (activations [feature, token]). Slices
are 512 tokens (PSUM bank limit); per-expert remainders become tail
slices kept >= ~196 wide so LDWEIGHTS stays hidden behind matmuls.
Everything is bf16 with fp32 PSUM accumulation; host sums partials in
fp32. Total per-core DMA ~111 MB vs ~1.15 ms of PE work — purely
tensor-engine-bound at ~99.9% theoretical PE occupancy.
"""

import numpy as np

H = 1024          # hidden dim
F = 3584          # FFN dim
E = 8             # experts
NF = 4            # FFN splits (cores 0-3 / 4-7 mirror over token halves)
FQ = F // NF      # 896 FFN columns per core
MFQ = FQ // 128   # 7 m-tiles over the FFN quarter
MH = H // 128     # 8 m-tiles over hidden (down-proj output)
KH = H // 128     # 8 k-tiles over hidden
NT = 512          # full token slice (psum bank = 512 fp32)

_compile_cache = {}
_last_result = None  # BassKernelResults of the most recent run (for profiling)


def _expert_slices(cap):
    """Slice widths for one expert's per-half token capacity `cap`.

    ceil(cap/512) slices of near-EQUAL width (multiples of 4, <= 512)
    rather than full 512s plus a narrow tail: all 8 cores run the same
    plan in lockstep, and a narrow slice doubles every core's LDWEIGHTS
    rate simultaneously, which trips the package power brake (measured
    +7-10 us gpio-throttle windows on the 224/280-wide tails). Equal
    widths keep the PE instruction rate uniform and every matmul wide
    enough (>=~400 rows) to fully hide its weight load."""
    if cap <= 0:
        return []
    n = -(-cap // NT)
    T = -(-cap // 4) * 4
    base = T // (4 * n) * 4
    rem = (T - base * n) // 4
    return [base + 4] * rem + [base] * (n - rem)


def _plan(counts):
    """Per-core slice plan: list of (expert, width). Identical for all
    cores; capacity per expert covers ceil(count/2) (the bigger half).
    The expert whose phase ends narrowest goes last so the kernel drains
    on the smallest final y transfer; the phase with the next-smallest
    tail goes FIRST with its slices reversed (narrow slice first), so the
    PE's first chain waits on a small x transfer and then runs gapless —
    a choppy first slice costs ~5 us of p-state re-ramps."""
    per_e = []
    for e, cnt in enumerate(counts):
        cap = -(-cnt // 2)
        ws = _expert_slices(cap)
        if ws:
            per_e.append((e, ws))
    per_e.sort(key=lambda p: -p[1][-1])
    if len(per_e) >= 2:
        first = per_e.pop(-2)
        per_e.insert(0, (first[0], first[1][::-1]))
    plan = []
    for e, ws in per_e:
        for w in ws:
            plan.append((e, w))
    return tuple(plan)


def _build(plan):
    """Build + compile the per-core Bass program for the given plan."""
    import concourse.bass as bass
    import concourse.mybir as mybir
    import concourse.tile as tile
    from concourse import bacc

    W = sum(w for _, w in plan)
    f32 = mybir.dt.float32
    bf16 = mybir.dt.bfloat16

    nc = bacc.Bacc("TRN2", target_bir_lowering=False, debug=False, num_devices=E)

    xT = nc.dram_tensor("xT", [H, W], bf16, kind="ExternalInput").ap()
    # per-expert merged w1|w3 quarter: col = m*2048 + which*1024 + k*128 + c
    w13s = nc.dram_tensor("w13s", [E, 128, MFQ * 2 * H], bf16, kind="ExternalInput").ap()
    # per-expert w2 quarter: col = mh*896 + kf*128 + c
    w2s = nc.dram_tensor("w2s", [E, 128, MH * FQ], bf16, kind="ExternalInput").ap()
    yT = nc.dram_tensor("yT", [H, W], bf16, kind="ExternalOutput").ap()

    xT_r = xT.rearrange("(k p) t -> p k t", p=128)
    yT_r = yT.rearrange("(m p) t -> p m t", p=128)

    # slice start offsets and the first slice index of each expert phase
    offs = []
    off = 0
    for e, w in plan:
        offs.append(off)
        off += w


    with tile.TileContext(nc, trace_sim=False) as tc:
        with (
            tc.tile_pool(name="xp", bufs=3) as xp,
            tc.tile_pool(name="w13p", bufs=3) as w13p,
            tc.tile_pool(name="w2p", bufs=3) as w2p,
            tc.tile_pool(name="hp", bufs=MFQ + 2) as hp,
            tc.tile_pool(name="hsp", bufs=3) as hsp,
            tc.tile_pool(name="yp", bufs=2) as yp,
            # 8 PSUM banks: mm2 chains are the tightest consumer-limited
            # rotation (psy WAR waits on the DVE copy two chains back at
            # bufs=2 — measured ~1-2 us stalls), so psy gets 3 banks;
            # ps3 is consumed by the mul immediately after its chain
            # stops, two iterations (~7 us) of slack, so 2 is enough.
            tc.tile_pool(name="ps1p", bufs=3, space="PSUM") as ps1p,
            tc.tile_pool(name="ps3p", bufs=2, space="PSUM") as ps3p,
            tc.tile_pool(name="psyp", bufs=3, space="PSUM") as psyp,
        ):
            w13_t = {}
            w2_t = {}

            def load_expert(e, split_first_m):
                """DMA one expert's w1|w3 quarter into a fresh tile; w2 is
                deferred to load_w2 (it isn't needed until the phase's mm2,
                and staggering it off the w13 burst eases the DMA rush that
                cost ~1.2 us at the early phase boundaries)."""
                w13t = w13p.tile([128, MFQ * 2 * H], bf16)
                if split_first_m:
                    # m=0's w1|w3 chunk first (512 KB, runs on a queue
                    # parallel to slice 0's x) so the first mm chain can
                    # start as soon as x lands
                    nc.sync.dma_start(w13t[:, : 2 * H], w13s[e][:, : 2 * H])
                else:
                    nc.sync.dma_start(w13t[:], w13s[e])
                w13_t[e] = (w13t, split_first_m)
                return w13t

            def load_w2(e):
                w2t = w2p.tile([128, MH * FQ], bf16)
                nc.sync.dma_start(w2t[:], w2s[e])
                w2_t[e] = w2t
                return w2t

            # startup: expert 0 m=0 weights, then the first x slice split
            # small-first, then the bulk of expert 0's weights
            e0 = plan[0][0]
            w13t0 = load_expert(e0, split_first_m=True)
            x_tiles = {}

            def load_x(si):
                e, width = plan[si]
                xt = xp.tile([128, KH, NT], bf16)
                o = offs[si]
                if si == 0:
                    nc.sync.dma_start(xt[:, :2, :width], xT_r[:, :2, o : o + width])
                    nc.sync.dma_start(xt[:, 2:, :width], xT_r[:, 2:, o : o + width])
                else:
                    nc.sync.dma_start(xt[:, :, :width], xT_r[:, :, o : o + width])
                x_tiles[si] = xt

            # slice 0's x in two half descriptors (parallel queues): the
            # first chain waits on the k<4 half, consumes it over ~0.7 us
            # while the k>=4 half lands — no mid-chain stall, but the PE
            # starts ~1 us sooner than a single-descriptor wait
            w0 = plan[0][1]
            xt0 = xp.tile([128, KH, NT], bf16)
            x_tiles[0] = xt0
            nc.sync.dma_start(xt0[:, :4, :w0], xT_r[:, :4, :w0])
            nc.sync.dma_start(xt0[:, 4:, :w0], xT_r[:, 4:, :w0])
            # per-m chunks so slice 0's mm chain m only waits on its own
            # 512 KB, not the whole 3.7 MB bulk (measured 3.9 us stall)
            for m in range(1, MFQ):
                nc.sync.dma_start(
                    w13t0[:, m * 2 * H : (m + 1) * 2 * H],
                    w13s[e0][:, m * 2 * H : (m + 1) * 2 * H],
                )
            load_w2(e0)
            if len(plan) > 1:
                load_x(1)

            for si, (e, width) in enumerate(plan):
                # prefetch: upcoming expert's w13 two slices ahead, its w2
                # one slice ahead (w2 is consumed a whole mm1/mm3 phase
                # later, and staggering halves the boundary DMA burst)
                for nxt in (si + 1, si + 2):
                    if nxt < len(plan) and plan[nxt][0] not in w13_t:
                        load_expert(plan[nxt][0], split_first_m=False)
                if si + 1 < len(plan) and plan[si + 1][0] not in w2_t:
                    load_w2(plan[si + 1][0])
                if si + 2 < len(plan):
                    load_x(si + 2)

                w13t, _ = w13_t[e]
                w2t = w2_t[e]
                xt = x_tiles.pop(si)

                h_tiles = []
                for m in range(MFQ):
                    ps1 = ps1p.tile([128, NT], f32)
                    for k in range(KH):
                        nc.tensor.matmul(
                            ps1[:, :width],
                            w13t[:, m * 2 * H + k * 128 : m * 2 * H + k * 128 + 128],
                            xt[:, k, :width],
                            start=(k == 0),
                            stop=(k == KH - 1),
                        )
                    ps3 = ps3p.tile([128, NT], f32)
                    for k in range(KH):
                        nc.tensor.matmul(
                            ps3[:, :width],
                            w13t[
                                :,
                                m * 2 * H + H + k * 128 : m * 2 * H + H + k * 128 + 128,
                            ],
                            xt[:, k, :width],
                            start=(k == 0),
                            stop=(k == KH - 1),
                        )
                    hs = hsp.tile([128, NT], f32)
                    nc.scalar.activation(
                        hs[:, :width], ps1[:, :width],
                        mybir.ActivationFunctionType.Silu,
                    )
                    ht = hp.tile([128, NT], bf16)
                    nc.vector.tensor_mul(ht[:, :width], hs[:, :width], ps3[:, :width])
                    h_tiles.append(ht)

                o = offs[si]
                # one [128, MH, width] y tile per slice, written per-mh by
                # Act-engine copies, shipped by a single 3D DMA: 1 output
                # descriptor per slice instead of 8, and the copy WAR gate
                # sits a whole slice back instead of 3 chains back (the
                # per-mh rotation deadlocked Act's in-order queue against
                # the PE's psy rotation for ~2-4 us per phase tail).
                yt = yp.tile([128, MH, NT], bf16)
                for mh in range(MH):
                    psy = psyp.tile([128, NT], f32)
                    for kf in range(MFQ):
                        nc.tensor.matmul(
                            psy[:, :width],
                            w2t[:, mh * FQ + kf * 128 : mh * FQ + kf * 128 + 128],
                            h_tiles[kf][:, :width],
                            start=(kf == 0),
                            stop=(kf == MFQ - 1),
                        )
                    # Copy on the Act engine (same act table as Silu, no
                    # reload) — the DVE's muls alone fill its queue on
                    # narrow tail slices; Act idles during mm2.
                    nc.scalar.activation(
                        yt[:, mh, :width], psy[:, :width],
                        mybir.ActivationFunctionType.Copy,
                    )
                if si == len(plan) - 1:
                    # final slice: ship each mh as soon as it's copied so
                    # the drain isn't gated on all 8 chains + one big DMA
                    for mh in range(MH):
                        nc.sync.dma_start(
                            yT_r[:, mh, o : o + width], yt[:, mh, :width]
                        )
                else:
                    nc.sync.dma_start(yT_r[:, :, o : o + width], yt[:, :, :width])

    nc.compile()
    return nc


def _route(x, gate_w, gate_b):
    """Host router: top-2 expert ids + renormalized combine weights."""
    logits = x.astype(np.float32) @ gate_w.astype(np.float32).T + gate_b.astype(
        np.float32
    )
    # top-2 by prob == top-2 by logit (softmax is monotonic); stable sort
    # matches jax.lax.top_k's lower-index-first tie-breaking.
    top2 = np.argsort(-logits, axis=-1, kind="stable")[:, :2]
    l2 = np.take_along_axis(logits, top2, axis=1)
    e2 = np.exp(l2 - l2.max(axis=1, keepdims=True))
    wts = e2 / e2.sum(axis=1, keepdims=True)
    return top2, wts.astype(np.float32)


def kernel(x, gate_w, gate_b, w1, w3, w2):
    import ml_dtypes
    from concourse.bass_utils import run_bass_kernel_spmd

    bf16 = ml_dtypes.bfloat16
    x = np.asarray(x, dtype=np.float32)
    T = x.shape[0]
    top2, wts = _route(x, np.asarray(gate_w), np.asarray(gate_b))

    idx_list, scale_list = [], []
    for e in range(E):
        sel = top2 == e                      # [T, 2] bool
        tok = np.nonzero(sel.any(axis=1))[0]
        idx_list.append(tok)
        # each token picks an expert at most once, so this take is unique
        which = sel[tok, 1].astype(np.int64)  # 0 if slot0, 1 if slot1
        scale_list.append(wts[tok, which])

    counts = [len(i) for i in idx_list]
    plan = _plan(counts)
    W = sum(w for _, w in plan)

    nc = _compile_cache.get(plan)
    if nc is None:
        nc = _build(plan)
        _compile_cache[plan] = nc

    w1 = np.asarray(w1, dtype=np.float32)
    w3 = np.asarray(w3, dtype=np.float32)
    w2 = np.asarray(w2, dtype=np.float32)
    x_bf = x.astype(bf16)

    # Per-expert lhsT layouts (as in the expert-parallel kernel):
    # W[k*128+p, m*128+c] -> [m, p, k*128+c] so each [128, H] tile row is
    # the stationary operand for one (m, k) matmul.
    w13_f = []  # per F-quarter: [E, 128, MFQ*2*H]
    w2_f = []   # per F-quarter: [E, 128, MH*FQ]
    MF = F // 128
    for f in range(NF):
        w13_f.append(np.empty((E, 128, MFQ * 2 * H), bf16))
        w2_f.append(np.empty((E, 128, MH * FQ), bf16))
    for e in range(E):
        w1s_e = (
            w1[e].reshape(KH, 128, MF, 128).transpose(2, 1, 0, 3).reshape(MF, 128, H)
        ).astype(bf16)
        w3s_e = (
            w3[e].reshape(KH, 128, MF, 128).transpose(2, 1, 0, 3).reshape(MF, 128, H)
        ).astype(bf16)
        w2s_e = (
            w2[e].reshape(MF, 128, MH, 128).transpose(2, 1, 0, 3).reshape(MH, 128, F)
        ).astype(bf16)
        for f in range(NF):
            q13 = np.stack(
                [w1s_e[f * MFQ : (f + 1) * MFQ], w3s_e[f * MFQ : (f + 1) * MFQ]],
                axis=1,
            )  # [MFQ, 2, 128, H]
            w13_f[f][e] = q13.transpose(2, 0, 1, 3).reshape(128, MFQ * 2 * H)
            w2_f[f][e] = (
                w2s_e[:, :, f * FQ : (f + 1) * FQ]
                .transpose(1, 0, 2)
                .reshape(128, MH * FQ)
            )

    # token streams per half: expert e's slice group covers tokens
    # [half*a_e ... ) where a_e = ceil(count/2); same column layout both halves
    slice_offs = []
    off = 0
    for e, w in plan:
        slice_offs.append(off)
        off += w
    halves = []  # per half: (xT, [(expert, tok_chunk, col_base, n)])
    for half in range(2):
        xTe = np.zeros((H, W), bf16)
        chunks = []
        used = {e: 0 for e in range(E)}
        for si, (e, width) in enumerate(plan):
            a = -(-counts[e] // 2)
            lo = half * a
            hi = min(counts[e], lo + a)
            start = lo + used[e]
            n = max(0, min(width, hi - start))
            used[e] += n
            if n:
                tok = idx_list[e][start : start + n]
                xTe[:, slice_offs[si] : slice_offs[si] + n] = x_bf[tok].T
                chunks.append((e, tok, slice_offs[si], n, start))
        halves.append((xTe, chunks))

    in_maps = []
    for c in range(E):
        f, half = c % NF, c // NF
        in_maps.append(
            {"xT": halves[half][0], "w13s": w13_f[f], "w2s": w2_f[f]}
        )

    global _last_result
    res = run_bass_kernel_spmd(nc, in_maps, core_ids=list(range(E)))
    _last_result = res

    out = np.zeros((T, H), np.float32)
    for half in range(2):
        ysum = np.zeros((H, W), np.float32)
        for f in range(NF):
            ysum += res.results[half * NF + f]["yT"].astype(np.float32)
        for e, tok, col, n, start in halves[half][1]:
            out[tok] += ysum[:, col : col + n].T * scale_list[e][start : start + n][
                :, None
            ]
    return out


# revision 28
# speedup vs baseline: 1.0400x; 1.0400x over previous
"""Mixtral-style sparse MoE block on 8 Trainium2 NeuronCores.

Strategy: 2D sharding — 4 FFN-quarters x 2 token-halves — all-bf16.
The router (tiny: T x H @ H x E) runs on the host as part of input
sharding. Each expert's routed tokens are split into two halves; cores
0-3 process half A and cores 4-7 process half B, with core (f, half)
computing only FFN columns [f*896, (f+1)*896). Every core therefore
processes ALL 8 experts over ~2048 tokens each, which balances the PE
load to within one token of the 8-core optimum regardless of routing
skew (the old expert-per-core layout ate the full max-expert imbalance,
~4%). The host sums the 4 FFN-quarter partial outputs per half, applies
the renormalized top-2 combine weights, and scatter-adds into [T, H].

Per-core program: 8 expert phases. Phase e holds w1/w3 (quarter, merged
into one [128, 7*2048] tile) and w2 (quarter, [128, 8*896]) in SBUF —
5.5 MB per expert, DMA'd once and prefetched one expert ahead — and
runs that expert's token slices:
    h  = silu(x @ w1q) * (x @ w3q)    # [C, 896]
    y += h @ w2q                      # [C, 1024] partial over F
in feature-on-partition layout (activations [feature, token]). Each
phase is cut into ceil(cap/512) near-EQUAL-width slices (<= 512, the
PSUM bank limit, >= ~400 in practice): all cores run the plan in
lockstep, and narrow tail slices doubled every core's LDWEIGHTS rate
simultaneously, tripping the package power brake for 7-10 us a pop.
Per-slice outputs ship as one 3D-descriptor DMA from a [128, 8, 512]
staging tile; psum->staging copies run on the Act engine. Everything is
bf16 with fp32 PSUM accumulation; host sums partials in fp32. Total
per-core DMA ~111 MB vs ~1.15 ms of PE work — purely tensor-engine-
bound at ~98% of the 8-core bf16 flops bound.
"""

import numpy as np

H = 1024          # hidden dim
F = 3584          # FFN dim
E = 8             # experts
NF = 4            # FFN splits (cores 0-3 / 4-7 mirror over token halves)
FQ = F // NF      # 896 FFN columns per core
MFQ = FQ // 128   # 7 m-tiles over the FFN quarter
MH = H // 128     # 8 m-tiles over hidden (down-proj output)
KH = H // 128     # 8 k-tiles over hidden
NT = 512          # full token slice (psum bank = 512 fp32)

_compile_cache = {}
_last_result = None  # BassKernelResults of the most recent run (for profiling)


def _expert_slices(cap):
    """Slice widths for one expert's per-half token capacity `cap`.

    ceil(cap/512) slices of near-EQUAL width (multiples of 4, <= 512)
    rather than full 512s plus a narrow tail: all 8 cores run the same
    plan in lockstep, and a narrow slice doubles every core's LDWEIGHTS
    rate simultaneously, which trips the package power brake (measured
    +7-10 us gpio-throttle windows on the 224/280-wide tails). Equal
    widths keep the PE instruction rate uniform and every matmul wide
    enough (>=~400 rows) to fully hide its weight load."""
    if cap <= 0:
        return []
    n = -(-cap // NT)
    T = -(-cap // 4) * 4
    base = T // (4 * n) * 4
    rem = (T - base * n) // 4
    return [base + 4] * rem + [base] * (n - rem)


def _plan(counts):
    """Per-core slice plan: list of (expert, width). Identical for all
    cores; capacity per expert covers ceil(count/2) (the bigger half).
    The expert whose phase ends narrowest goes last so the kernel drains
    on the smallest final y transfer; the phase with the next-smallest
    tail goes FIRST with its slices reversed (narrow slice first), so the
    PE's first chain waits on a small x transfer and then runs gapless —
    a choppy first slice costs ~5 us of p-state re-ramps."""
    per_e = []
    for e, cnt in enumerate(counts):
        cap = -(-cnt // 2)
        ws = _expert_slices(cap)
        if ws:
            per_e.append((e, ws))
    per_e.sort(key=lambda p: -p[1][-1])
    if len(per_e) >= 2:
        first = per_e.pop(-2)
        per_e.insert(0, (first[0], first[1][::-1]))
    plan = []
    for e, ws in per_e:
        for w in ws:
            plan.append((e, w))
    return tuple(plan)


def _build(plan):
    """Build + compile the per-core Bass program for the given plan."""
    import concourse.bass as bass
    import concourse.mybir as mybir
    import concourse.tile as tile
    from concourse import bacc

    W = sum(w for _, w in plan)
    f32 = mybir.dt.float32
    bf16 = mybir.dt.bfloat16

    nc = bacc.Bacc("TRN2", target_bir_lowering=False, debug=False, num_devices=E)

    xT = nc.dram_tensor("xT", [H, W], bf16, kind="ExternalInput").ap()
    # per-expert merged w1|w3 quarter: col = m*2048 + which*1024 + k*128 + c
    w13s = nc.dram_tensor("w13s", [E, 128, MFQ * 2 * H], bf16, kind="ExternalInput").ap()
    # per-expert w2 quarter: col = mh*896 + kf*128 + c
    w2s = nc.dram_tensor("w2s", [E, 128, MH * FQ], bf16, kind="ExternalInput").ap()
    yT = nc.dram_tensor("yT", [H, W], bf16, kind="ExternalOutput").ap()

    xT_r = xT.rearrange("(k p) t -> p k t", p=128)
    yT_r = yT.rearrange("(m p) t -> p m t", p=128)

    # slice start offsets and the first slice index of each expert phase
    offs = []
    off = 0
    for e, w in plan:
        offs.append(off)
        off += w


    with tile.TileContext(nc, trace_sim=False) as tc:
        with (
            tc.tile_pool(name="xp", bufs=3) as xp,
            tc.tile_pool(name="w13p", bufs=3) as w13p,
            tc.tile_pool(name="w2p", bufs=3) as w2p,
            tc.tile_pool(name="hp", bufs=MFQ + 2) as hp,
            tc.tile_pool(name="hsp", bufs=3) as hsp,
            tc.tile_pool(name="yp", bufs=2) as yp,
            # 8 PSUM banks: mm2 chains are the tightest consumer-limited
            # rotation (psy WAR waits on the DVE copy two chains back at
            # bufs=2 — measured ~1-2 us stalls), so psy gets 3 banks;
            # ps3 is consumed by the mul immediately after its chain
            # stops, two iterations (~7 us) of slack, so 2 is enough.
            tc.tile_pool(name="ps1p", bufs=3, space="PSUM") as ps1p,
            tc.tile_pool(name="ps3p", bufs=2, space="PSUM") as ps3p,
            tc.tile_pool(name="psyp", bufs=3, space="PSUM") as psyp,
        ):
            w13_t = {}
            w2_t = {}

            def load_expert(e, split_first_m):
                """DMA one expert's w1|w3 quarter into a fresh tile; w2 is
                deferred to load_w2 (it isn't needed until the phase's mm2,
                and staggering it off the w13 burst eases the DMA rush that
                cost ~1.2 us at the early phase boundaries)."""
                w13t = w13p.tile([128, MFQ * 2 * H], bf16)
                if split_first_m:
                    # m=0's w1|w3 chunk first (512 KB, runs on a queue
                    # parallel to slice 0's x) so the first mm chain can
                    # start as soon as x lands
                    nc.sync.dma_start(w13t[:, : 2 * H], w13s[e][:, : 2 * H])
                else:
                    nc.sync.dma_start(w13t[:], w13s[e])
                w13_t[e] = (w13t, split_first_m)
                return w13t

            def load_w2(e):
                w2t = w2p.tile([128, MH * FQ], bf16)
                nc.sync.dma_start(w2t[:], w2s[e])
                w2_t[e] = w2t
                return w2t

            # startup: expert 0 m=0 weights, then the first x slice split
            # small-first, then the bulk of expert 0's weights
            e0 = plan[0][0]
            w13t0 = load_expert(e0, split_first_m=True)
            x_tiles = {}

            def load_x(si):
                e, width = plan[si]
                xt = xp.tile([128, KH, NT], bf16)
                o = offs[si]
                if si == 0:
                    nc.sync.dma_start(xt[:, :2, :width], xT_r[:, :2, o : o + width])
                    nc.sync.dma_start(xt[:, 2:, :width], xT_r[:, 2:, o : o + width])
                else:
                    nc.sync.dma_start(xt[:, :, :width], xT_r[:, :, o : o + width])
                x_tiles[si] = xt

            # slice 0's x as ONE descriptor: the first chain waits for the
            # whole slice and then runs without the mid-chain DMA stalls
            # that reset the PE p-state ramp (splitting it was measured
            # neutral-to-worse — the halves can land on one queue)
            w0 = plan[0][1]
            xt0 = xp.tile([128, KH, NT], bf16)
            x_tiles[0] = xt0
            nc.sync.dma_start(xt0[:, :, :w0], xT_r[:, :, :w0])
            # per-m chunks so slice 0's mm chain m only waits on its own
            # 512 KB, not the whole 3.7 MB bulk (measured 3.9 us stall)
            for m in range(1, MFQ):
                nc.sync.dma_start(
                    w13t0[:, m * 2 * H : (m + 1) * 2 * H],
                    w13s[e0][:, m * 2 * H : (m + 1) * 2 * H],
                )
            load_w2(e0)
            if len(plan) > 1:
                load_x(1)

            for si, (e, width) in enumerate(plan):
                # prefetch: upcoming expert's w13 two slices ahead, its w2
                # one slice ahead (w2 is consumed a whole mm1/mm3 phase
                # later, and staggering halves the boundary DMA burst)
                for nxt in (si + 1, si + 2):
                    if nxt < len(plan) and plan[nxt][0] not in w13_t:
                        load_expert(plan[nxt][0], split_first_m=False)
                if si + 1 < len(plan) and plan[si + 1][0] not in w2_t:
                    load_w2(plan[si + 1][0])
                if si + 2 < len(plan):
                    load_x(si + 2)

                w13t, _ = w13_t[e]
                w2t = w2_t[e]
                xt = x_tiles.pop(si)

                h_tiles = []
                for m in range(MFQ):
                    ps1 = ps1p.tile([128, NT], f32)
                    for k in range(KH):
                        nc.tensor.matmul(
                            ps1[:, :width],
                            w13t[:, m * 2 * H + k * 128 : m * 2 * H + k * 128 + 128],
                            xt[:, k, :width],
                            start=(k == 0),
                            stop=(k == KH - 1),
                        )
                    ps3 = ps3p.tile([128, NT], f32)
                    for k in range(KH):
                        nc.tensor.matmul(
                            ps3[:, :width],
                            w13t[
                                :,
                                m * 2 * H + H + k * 128 : m * 2 * H + H + k * 128 + 128,
                            ],
                            xt[:, k, :width],
                            start=(k == 0),
                            stop=(k == KH - 1),
                        )
                    hs = hsp.tile([128, NT], f32)
                    nc.scalar.activation(
                        hs[:, :width], ps1[:, :width],
                        mybir.ActivationFunctionType.Silu,
                    )
                    ht = hp.tile([128, NT], bf16)
                    nc.vector.tensor_mul(ht[:, :width], hs[:, :width], ps3[:, :width])
                    h_tiles.append(ht)

                o = offs[si]
                # one [128, MH, width] y tile per slice, written per-mh by
                # Act-engine copies, shipped by a single 3D DMA: 1 output
                # descriptor per slice instead of 8, and the copy WAR gate
                # sits a whole slice back instead of 3 chains back (the
                # per-mh rotation deadlocked Act's in-order queue against
                # the PE's psy rotation for ~2-4 us per phase tail).
                yt = yp.tile([128, MH, NT], bf16)
                for mh in range(MH):
                    psy = psyp.tile([128, NT], f32)
                    for kf in range(MFQ):
                        nc.tensor.matmul(
                            psy[:, :width],
                            w2t[:, mh * FQ + kf * 128 : mh * FQ + kf * 128 + 128],
                            h_tiles[kf][:, :width],
                            start=(kf == 0),
                            stop=(kf == MFQ - 1),
                        )
                    # Copy on the Act engine (same act table as Silu, no
                    # reload) — the DVE's muls alone fill its queue on
                    # narrow tail slices; Act idles during mm2.
                    nc.scalar.activation(
                        yt[:, mh, :width], psy[:, :width],
                        mybir.ActivationFunctionType.Copy,
                    )
                if si == len(plan) - 1:
                    # final slice: ship each mh as soon as it's copied so
                    # the drain isn't gated on all 8 chains + one big DMA
                    for mh in range(MH):
                        nc.sync.dma_start(
                            yT_r[:, mh, o : o + width], yt[:, mh, :width]
                        )
                else:
                    nc.sync.dma_start(yT_r[:, :, o : o + width], yt[:, :, :width])

    nc.compile()
    return nc


def _route(x, gate_w, gate_b):
    """Host router: top-2 expert ids + renormalized combine weights."""
    logits = x.astype(np.float32) @ gate_w.astype(np.float32).T + gate_b.astype(
        np.float32
    )
    # top-2 by prob == top-2 by logit (softmax is monotonic); stable sort
    # matches jax.lax.top_k's lower-index-first tie-breaking.
    top2 = np.argsort(-logits, axis=-1, kind="stable")[:, :2]
    l2 = np.take_along_axis(logits, top2, axis=1)
    e2 = np.exp(l2 - l2.max(axis=1, keepdims=True))
    wts = e2 / e2.sum(axis=1, keepdims=True)
    return top2, wts.astype(np.float32)


def kernel(x, gate_w, gate_b, w1, w3, w2):
    import ml_dtypes
    from concourse.bass_utils import run_bass_kernel_spmd

    bf16 = ml_dtypes.bfloat16
    x = np.asarray(x, dtype=np.float32)
    T = x.shape[0]
    top2, wts = _route(x, np.asarray(gate_w), np.asarray(gate_b))

    idx_list, scale_list = [], []
    for e in range(E):
        sel = top2 == e                      # [T, 2] bool
        tok = np.nonzero(sel.any(axis=1))[0]
        idx_list.append(tok)
        # each token picks an expert at most once, so this take is unique
        which = sel[tok, 1].astype(np.int64)  # 0 if slot0, 1 if slot1
        scale_list.append(wts[tok, which])

    counts = [len(i) for i in idx_list]
    plan = _plan(counts)
    W = sum(w for _, w in plan)

    nc = _compile_cache.get(plan)
    if nc is None:
        nc = _build(plan)
        _compile_cache[plan] = nc

    w1 = np.asarray(w1, dtype=np.float32)
    w3 = np.asarray(w3, dtype=np.float32)
    w2 = np.asarray(w2, dtype=np.float32)
    x_bf = x.astype(bf16)

    # Per-expert lhsT layouts (as in the expert-parallel kernel):
    # W[k*128+p, m*128+c] -> [m, p, k*128+c] so each [128, H] tile row is
    # the stationary operand for one (m, k) matmul.
    w13_f = []  # per F-quarter: [E, 128, MFQ*2*H]
    w2_f = []   # per F-quarter: [E, 128, MH*FQ]
    MF = F // 128
    for f in range(NF):
        w13_f.append(np.empty((E, 128, MFQ * 2 * H), bf16))
        w2_f.append(np.empty((E, 128, MH * FQ), bf16))
    for e in range(E):
        w1s_e = (
            w1[e].reshape(KH, 128, MF, 128).transpose(2, 1, 0, 3).reshape(MF, 128, H)
        ).astype(bf16)
        w3s_e = (
            w3[e].reshape(KH, 128, MF, 128).transpose(2, 1, 0, 3).reshape(MF, 128, H)
        ).astype(bf16)
        w2s_e = (
            w2[e].reshape(MF, 128, MH, 128).transpose(2, 1, 0, 3).reshape(MH, 128, F)
        ).astype(bf16)
        for f in range(NF):
            q13 = np.stack(
                [w1s_e[f * MFQ : (f + 1) * MFQ], w3s_e[f * MFQ : (f + 1) * MFQ]],
                axis=1,
            )  # [MFQ, 2, 128, H]
            w13_f[f][e] = q13.transpose(2, 0, 1, 3).reshape(128, MFQ * 2 * H)
            w2_f[f][e] = (
                w2s_e[:, :, f * FQ : (f + 1) * FQ]
                .transpose(1, 0, 2)
                .reshape(128, MH * FQ)
            )

    # token streams per half: expert e's slice group covers tokens
    # [half*a_e ... ) where a_e = ceil(count/2); same column layout both halves
    slice_offs = []
    off = 0
    for e, w in plan:
        slice_offs.append(off)
        off += w
    halves = []  # per half: (xT, [(expert, tok_chunk, col_base, n)])
    for half in range(2):
        xTe = np.zeros((H, W), bf16)
        chunks = []
        used = {e: 0 for e in range(E)}
        for si, (e, width) in enumerate(plan):
            a = -(-counts[e] // 2)
            lo = half * a
            hi = min(counts[e], lo + a)
            start = lo + used[e]
            n = max(0, min(width, hi - start))
            used[e] += n
            if n:
                tok = idx_list[e][start : start + n]
                xTe[:, slice_offs[si] : slice_offs[si] + n] = x_bf[tok].T
                chunks.append((e, tok, slice_offs[si], n, start))
        halves.append((xTe, chunks))

    in_maps = []
    for c in range(E):
        f, half = c % NF, c // NF
        in_maps.append(
            {"xT": halves[half][0], "w13s": w13_f[f], "w2s": w2_f[f]}
        )

    global _last_result
    res = run_bass_kernel_spmd(nc, in_maps, core_ids=list(range(E)))
    _last_result = res

    out = np.zeros((T, H), np.float32)
    for half in range(2):
        ysum = np.zeros((H, W), np.float32)
        for f in range(NF):
            ysum += res.results[half * NF + f]["yT"].astype(np.float32)
        for e, tok, col, n, start in halves[half][1]:
            out[tok] += ysum[:, col : col + n].T * scale_list[e][start : start + n][
                :, None
            ]
    return out
